# revision 1
# baseline (speedup 1.0000x reference)
"""Trainium2 Bass kernel for: Conv3d(3->16, k=3x3x3, VALID) + bias -> min over
depth -> softmax over channels.

Input  x: (16, 3, 32, 128, 128) f32   [N, C_in, D, H, W]
Weight w: (16, 3, 3, 3, 3) f32        [C_out, C_in, kD, kH, kW]
Bias   b: (16,) f32
Output  : (16, 16, 126, 126) f32      [N, C_out, H_out, W_out]

Data-parallel over batch: 2 batches per core x 8 cores. Per core:

  - x stored per (batch, h-half) as quad tiles: strip r (32-partition aligned)
    holds 12 rows = (d-window of 4) x (ci 3) for depth-output pair d0=8g+2r,
    free dim = local (h, w) flattened (66 or 64 h-rows).
  - Conv as 16-way tile_position-packed matmuls: tile (r,c) = [K=12, M=32,
    N=512], M = (delta 2 douts x 16 co), 9 accumulating MMs over (kh,kw) with
    free-dim shifted rhs. Weight block[(dl,ci),(delta,co)] = w[co,ci,dl-delta,
    kh,kw]. PSUM supertile [128, 4*512]: partition = (c 4 chunks x delta x co),
    free = (pair r x 512 spatial).
  - Depth-min: DVE tensor_tensor(min) [128, npairs*512] psum vs running SBUF
    buffer (accumulated over quad groups g), then tensor_reduce(min) over the
    pair slots (free axis), then copy-DMA + accum_op=min DMA collapses delta.
  - Softmax over co: ACT exp (bias fused; min(y)+b == min(y+b)), PE
    ones-matmul for co-sums, DVE reciprocal, DMA broadcast, DVE multiply.
"""

import os
import sys

sys.path.insert(0, "/opt/trn_rl_repo")

import numpy as np

import concourse.bass as bass
import concourse.bacc as bacc
import concourse.tile as tile
import concourse.mybir as mybir
from concourse import bass_utils

F32 = mybir.dt.float32

N_CORES = 8
NB = 2           # batches per core
CI = 3
D = 32
H = 128
W = 128
CO = 16
CHUNK = 512
NGRP = 4         # pair-quad groups; g<3: 4 pairs, g=3: 3 pairs
NCGL = 4         # chunk groups per h-half (each 4 col-tiles x 512)
HOUT = 126
WOUT = 126
PAD = 320
QF = 66 * W + PAD  # quad tile free size (worst case hh=0)

_COMPILED = {}


def _pairs_in_group(g):
    return 4 if g < 3 else 3


def _build_weight_blocks(conv_weight):
    """[128, 288]: strip r rows 0..11 = (dl*3+ci); cols khw*32 + delta*16 + co
    = w[co, ci, dl-delta, kh, kw] (0 outside kd range). Replicated per strip."""
    blk = np.zeros((12, 9, 32), dtype=np.float32)
    for dl in range(4):
        for ci in range(CI):
            row = dl * 3 + ci
            for kh in range(3):
                for kw in range(3):
                    khw = kh * 3 + kw
                    for delta in range(2):
                        kd = dl - delta
                        if 0 <= kd <= 2:
                            blk[row, khw, delta * 16:(delta + 1) * 16] = \
                                conv_weight[:, ci, kd, kh, kw]
    w_sb = np.zeros((128, 288), dtype=np.float32)
    for r in range(4):
        w_sb[32 * r:32 * r + 12, :] = blk.reshape(12, 288)
    return w_sb


def _build_ones():
    """[128, 8]: col k*4+c sums partitions {k*64 + c*16 + co : co}."""
    ones = np.zeros((128, 8), dtype=np.float32)
    for p in range(128):
        k, c = p // 64, (p % 64) // 16
        ones[p, k * 4 + c] = 1.0
    return ones


def _build_ones_bc():
    """[8, 128]: transpose of _build_ones -- broadcasts row j over its
    16-partition group."""
    return np.ascontiguousarray(_build_ones().T)


def _build_bias128(conv_bias):
    """[128, 1]: partition k*64 + c*16 + co -> bias[co]."""
    b = np.zeros((128, 1), dtype=np.float32)
    for p in range(128):
        b[p, 0] = conv_bias[p % 16]
    return b


def _emit_kernel(tc):
    nc = tc.nc
    x_ap = nc.dram_tensor("x", [NB, D, CI, H, W], F32, kind="ExternalInput").ap()
    w_ap = nc.dram_tensor("w", [128, 288], F32, kind="ExternalInput").ap()
    bias_ap = nc.dram_tensor("bias", [128, 1], F32, kind="ExternalInput").ap()
    ones_ap = nc.dram_tensor("ones", [128, 8], F32, kind="ExternalInput").ap()
    ones_bc_ap = nc.dram_tensor("ones_bc", [8, 128], F32,
                                kind="ExternalInput").ap()
    y_ap = nc.dram_tensor("y", [NB, CO, HOUT, WOUT], F32,
                          kind="ExternalOutput").ap()

    from contextlib import ExitStack

    with ExitStack() as ctx:
        const_pool = ctx.enter_context(tc.tile_pool(name="const", bufs=1))
        in_pool = ctx.enter_context(tc.tile_pool(name="in", bufs=2))
        run_pool = ctx.enter_context(tc.tile_pool(name="run", bufs=1))
        sm_pool = ctx.enter_context(tc.tile_pool(name="sm", bufs=2))
        exp_pool = ctx.enter_context(tc.tile_pool(name="expp", bufs=5))
        psum_pool = ctx.enter_context(tc.tile_pool(name="ps", bufs=2,
                                                   space="PSUM"))

        w_sb = const_pool.tile([128, 288], F32, tag="w")
        nc.sync.dma_start(w_sb[:, :], w_ap[:, :])
        bias_sb = const_pool.tile([128, 1], F32, tag="bias")
        nc.sync.dma_start(bias_sb[:, :], bias_ap[:, :])
        ones_sb = const_pool.tile([128, 8], F32, tag="ones")
        nc.sync.dma_start(ones_sb[:, :], ones_ap[:, :])
        ones_bc_sb = const_pool.tile([8, 128], F32, tag="onesbc")
        nc.sync.dma_start(ones_bc_sb[:, :], ones_bc_ap[:, :])

        for n in range(NB):
            for hh in range(2):
                h0 = 64 * hh
                hrows = 66 if hh == 0 else 64
                running = [run_pool.tile([128, 4 * CHUNK], F32,
                                         name=f"run{cg}", tag=f"run{cg}")
                           for cg in range(NCGL)]
                for g in range(NGRP):
                    npairs = _pairs_in_group(g)
                    quad = in_pool.tile([128, QF], F32, tag="quad")
                    nc.gpsimd.memset(quad[:, hrows * W:QF], 0.0)
                    for r in range(npairs):
                        d0 = 8 * g + 2 * r
                        src = x_ap[n, d0:d0 + 4, :, h0:h0 + hrows, :].rearrange(
                            "d c h w -> (d c) (h w)")
                        nc.sync.dma_start(
                            quad[32 * r:32 * r + 12, 0:hrows * W], src)
                    for cgl in range(NCGL):
                        ps = psum_pool.tile([128, 4 * CHUNK], F32, tag="big")
                        for khw in range(9):
                            kh, kw = khw // 3, khw % 3
                            koff = kh * W + kw
                            for r in range(npairs):
                                for c in range(4):
                                    s0 = cgl * 2048 + c * CHUNK + koff
                                    nc.tensor.matmul(
                                        ps[32 * c:32 * c + 32,
                                           r * CHUNK:(r + 1) * CHUNK],
                                        lhsT=w_sb[32 * r:32 * r + 12,
                                                  khw * 32:(khw + 1) * 32],
                                        rhs=quad[32 * r:32 * r + 12,
                                                 s0:s0 + CHUNK],
                                        start=(khw == 0),
                                        stop=(khw == 8),
                                        tile_position=(32 * r, 32 * c),
                                        skip_group_check=True,
                                    )
                        width = npairs * CHUNK
                        if g == 0:
                            nc.vector.tensor_copy(running[cgl][:, 0:width],
                                                  ps[:, 0:width])
                        else:
                            nc.vector.tensor_tensor(
                                out=running[cgl][:, 0:width],
                                in0=ps[:, 0:width],
                                in1=running[cgl][:, 0:width],
                                op=mybir.AluOpType.min,
                            )
                # tail: two chunk groups merge into one [128, 4096] tile with
                # (delta, pair) in the free dim (DMA crosses partitions),
                # then one reduce_min + softmax on full 128 lanes.
                # t2 partitions: p = k*64 + c*16 + co (k = cgl parity).
                for cp in range(NCGL // 2):
                    t2 = sm_pool.tile([128, 8 * CHUNK], F32, tag="t2")
                    for k in range(2):
                        cgl = 2 * cp + k
                        for dlt in range(2):
                            for c in range(4):
                                nc.sync.dma_start(
                                    t2[64 * k + 16 * c:64 * k + 16 * c + 16,
                                       2048 * dlt:2048 * (dlt + 1)],
                                    running[cgl][32 * c + 16 * dlt:
                                                 32 * c + 16 * dlt + 16, :])
                    mins = sm_pool.tile([128, CHUNK], F32, tag="mins")
                    nc.vector.tensor_reduce(
                        out=mins[:, :],
                        in_=t2[:, :].rearrange("p (dr s) -> p s dr", s=CHUNK),
                        axis=mybir.AxisListType.X,
                        op=mybir.AluOpType.min,
                    )
                    expt = exp_pool.tile([128, CHUNK], F32, tag="exp")
                    nc.scalar.activation(
                        expt[:, :], mins[:, :],
                        mybir.ActivationFunctionType.Exp,
                        bias=bias_sb[:, :], scale=1.0)
                    esum = psum_pool.tile([8, CHUNK], F32, tag="big")
                    nc.tensor.matmul(esum[:, :], lhsT=ones_sb[:, :],
                                     rhs=expt[:, :], start=True, stop=True)
                    rec = sm_pool.tile([8, CHUNK], F32, tag="rec")
                    nc.vector.reciprocal(rec[:, :], esum[:, :])
                    # broadcast rec rows over their 16-co partition groups
                    # via a K=8 ones matmul (PE does the partition fan-out)
                    rb = psum_pool.tile([128, CHUNK], F32, tag="big")
                    nc.tensor.matmul(rb[:, :], lhsT=ones_bc_sb[:, :],
                                     rhs=rec[:, :], start=True, stop=True)
                    soft = sm_pool.tile([128, CHUNK], F32, tag="soft")
                    nc.vector.tensor_tensor(
                        out=soft[:, :], in0=rb[:, :], in1=expt[:, :],
                        op=mybir.AluOpType.mult)
                    # emit valid rows: h = h0 + (2cp+k)*16 + c*4 + i
                    for k in range(2):
                        hbase = h0 + (2 * cp + k) * 16
                        for c in range(4):
                            hc = hbase + 4 * c
                            ni = min(4, HOUT - hc)
                            if ni <= 0:
                                continue
                            src = soft[64 * k + 16 * c:64 * k + 16 * c + 16,
                                       :].rearrange(
                                "co (i w) -> co i w", i=4)[:, 0:ni, 0:WOUT]
                            dst = y_ap[n, :, hc:hc + ni, :]
                            nc.sync.dma_start(dst, src)


def _compile():
    if "nc" in _COMPILED:
        return _COMPILED["nc"]
    nc = bacc.Bacc("TRN2", target_bir_lowering=False, debug=False,
                   num_devices=N_CORES)
    with tile.TileContext(nc) as tc:
        _emit_kernel(tc)
    nc.compile()
    _COMPILED["nc"] = nc
    return nc


def kernel(x, conv_weight, conv_bias):
    x = np.asarray(x, dtype=np.float32)
    conv_weight = np.asarray(conv_weight, dtype=np.float32)
    conv_bias = np.asarray(conv_bias, dtype=np.float32)

    xp = np.ascontiguousarray(x.transpose(0, 2, 1, 3, 4))  # [N, D, C, H, W]
    w_sb = _build_weight_blocks(conv_weight)
    bias_sb = _build_bias128(conv_bias)
    ones_sb = _build_ones()

    nc = _compile()
    in_maps = []
    for i in range(N_CORES):
        in_maps.append({
            "x": np.ascontiguousarray(xp[NB * i:NB * (i + 1)]),
            "w": w_sb,
            "bias": bias_sb,
            "ones": ones_sb,
            "ones_bc": _build_ones_bc(),
        })
    res = bass_utils.run_bass_kernel_spmd(
        nc, in_maps, core_ids=list(range(N_CORES)),
        trace=bool(int(os.environ.get("KERNEL_TRACE", "0"))),
    )
    _COMPILED["last_results"] = res
    out = np.concatenate([res.results[i]["y"] for i in range(N_CORES)], axis=0)
    return out


if __name__ == "__main__":
    _compile()
    print("build OK")



# revision 5
# speedup vs baseline: 2.2506x; 2.2506x over previous
"""Trainium2 Bass kernel for: Conv3d(3->16, k=3x3x3, VALID) + bias -> min over
depth -> softmax over channels.

Input  x: (16, 3, 32, 128, 128) f32   [N, C_in, D, H, W]
Weight w: (16, 3, 3, 3, 3) f32        [C_out, C_in, kD, kH, kW]
Bias   b: (16,) f32
Output  : (16, 16, 126, 126) f32      [N, C_out, H_out, W_out]

Data-parallel over batch: 2 batches per core x 8 cores. Per core:

  - x stored per (batch, h-half) as quad tiles: strip r (32-partition aligned)
    holds 12 rows = (d-window of 4) x (ci 3) for depth-output pair d0=8g+2r,
    free dim = local (h, w) flattened (66 or 64 h-rows).
  - Conv as 16-way tile_position-packed matmuls: tile (r,c) = [K=12, M=32,
    N=512], M = (delta 2 douts x 16 co), 9 accumulating MMs over (kh,kw) with
    free-dim shifted rhs. Weight block[(dl,ci),(delta,co)] = w[co,ci,dl-delta,
    kh,kw]. PSUM supertile [128, 4*512]: partition = (c 4 chunks x delta x co),
    free = (pair r x 512 spatial).
  - Depth-min: DVE tensor_tensor(min) [128, npairs*512] psum vs running SBUF
    buffer (accumulated over quad groups g), then tensor_reduce(min) over the
    pair slots (free axis), then copy-DMA + accum_op=min DMA collapses delta.
  - Softmax over co: ACT exp (bias fused; min(y)+b == min(y+b)), PE
    ones-matmul for co-sums, DVE reciprocal, DMA broadcast, DVE multiply.
"""

import os
import sys

sys.path.insert(0, "/opt/trn_rl_repo")

import numpy as np
import ml_dtypes

import concourse.bass as bass
import concourse.bacc as bacc
import concourse.tile as tile
import concourse.mybir as mybir
from concourse import bass_utils

F32 = mybir.dt.float32
BF16 = mybir.dt.bfloat16

N_CORES = 8
NB = 2           # batches per core
CI = 3
D = 32
H = 128
W = 128
CO = 16
CHUNK = 512
NGRP = 4         # pair-quad groups; g<3: 4 pairs, g=3: 3 pairs
NCGL = 4         # chunk groups per h-half (each 4 col-tiles x 512)
HOUT = 126
WOUT = 126
PAD = 320
QF = 66 * W + PAD  # quad tile free size (worst case hh=0)

_COMPILED = {}


def _pairs_in_group(g):
    return 4 if g < 3 else 3


def _build_weight_blocks(conv_weight):
    """[128, 288]: strip r rows 0..11 = (dl*3+ci); cols khw*32 + delta*16 + co
    = w[co, ci, dl-delta, kh, kw] (0 outside kd range). Replicated per strip."""
    blk = np.zeros((12, 9, 32), dtype=np.float32)
    for dl in range(4):
        for ci in range(CI):
            row = dl * 3 + ci
            for kh in range(3):
                for kw in range(3):
                    khw = kh * 3 + kw
                    for delta in range(2):
                        kd = dl - delta
                        if 0 <= kd <= 2:
                            blk[row, khw, delta * 16:(delta + 1) * 16] = \
                                conv_weight[:, ci, kd, kh, kw]
    w_sb = np.zeros((128, 288), dtype=np.float32)
    for r in range(4):
        w_sb[32 * r:32 * r + 12, :] = blk.reshape(12, 288)
    return w_sb


def _build_ones():
    """[128, 8]: col k*4+c sums partitions {k*64 + c*16 + co : co}."""
    ones = np.zeros((128, 8), dtype=np.float32)
    for p in range(128):
        k, c = p // 64, (p % 64) // 16
        ones[p, k * 4 + c] = 1.0
    return ones


def _build_ones_bc():
    """[8, 128]: transpose of _build_ones -- broadcasts row j over its
    16-partition group."""
    return np.ascontiguousarray(_build_ones().T)


def _build_bias128(conv_bias):
    """[128, 1]: partition k*64 + c*16 + co -> bias[co]."""
    b = np.zeros((128, 1), dtype=np.float32)
    for p in range(128):
        b[p, 0] = conv_bias[p % 16]
    return b


def _emit_kernel(tc):
    nc = tc.nc
    x_ap = nc.dram_tensor("x", [NB, D, CI, H, W], BF16, kind="ExternalInput").ap()
    w_ap = nc.dram_tensor("w", [128, 288], BF16, kind="ExternalInput").ap()
    bias_ap = nc.dram_tensor("bias", [128, 1], F32, kind="ExternalInput").ap()
    ones_ap = nc.dram_tensor("ones", [128, 8], F32, kind="ExternalInput").ap()
    ones_bc_ap = nc.dram_tensor("ones_bc", [8, 128], F32,
                                kind="ExternalInput").ap()
    y_ap = nc.dram_tensor("y", [NB, CO, HOUT, WOUT], F32,
                          kind="ExternalOutput").ap()

    from contextlib import ExitStack

    with ExitStack() as ctx:
        const_pool = ctx.enter_context(tc.tile_pool(name="const", bufs=1))
        in_pool = ctx.enter_context(tc.tile_pool(name="in", bufs=2))
        run_pool = ctx.enter_context(tc.tile_pool(name="run", bufs=1))
        sm_pool = ctx.enter_context(tc.tile_pool(name="sm", bufs=2))
        exp_pool = ctx.enter_context(tc.tile_pool(name="expp", bufs=5))
        psum_pool = ctx.enter_context(tc.tile_pool(name="ps", bufs=2,
                                                   space="PSUM"))

        w_sb = const_pool.tile([128, 288], BF16, tag="w")
        nc.sync.dma_start(w_sb[:, :], w_ap[:, :])
        bias_sb = const_pool.tile([128, 1], F32, tag="bias")
        nc.sync.dma_start(bias_sb[:, :], bias_ap[:, :])
        ones_sb = const_pool.tile([128, 8], F32, tag="ones")
        nc.sync.dma_start(ones_sb[:, :], ones_ap[:, :])
        ones_bc_sb = const_pool.tile([8, 128], F32, tag="onesbc")
        nc.sync.dma_start(ones_bc_sb[:, :], ones_bc_ap[:, :])

        for n in range(NB):
            for hh in range(2):
                h0 = 64 * hh
                hrows = 66 if hh == 0 else 64
                running = [run_pool.tile([128, 4 * CHUNK], F32,
                                         name=f"run{cg}", tag=f"run{cg}")
                           for cg in range(NCGL)]
                for g in range(NGRP):
                    npairs = _pairs_in_group(g)
                    quad = in_pool.tile([128, QF], BF16, tag="quad")
                    nc.gpsimd.memset(quad[:, hrows * W:QF], 0.0)
                    for r in range(npairs):
                        d0 = 8 * g + 2 * r
                        src = x_ap[n, d0:d0 + 4, :, h0:h0 + hrows, :].rearrange(
                            "d c h w -> (d c) (h w)")
                        nc.sync.dma_start(
                            quad[32 * r:32 * r + 12, 0:hrows * W], src)
                    for cgl in range(NCGL):
                        ps = psum_pool.tile([128, 4 * CHUNK], F32, tag="big")
                        for khw in range(9):
                            kh, kw = khw // 3, khw % 3
                            koff = kh * W + kw
                            for r in range(npairs):
                                for c in range(4):
                                    s0 = cgl * 2048 + c * CHUNK + koff
                                    nc.tensor.matmul(
                                        ps[32 * c:32 * c + 32,
                                           r * CHUNK:(r + 1) * CHUNK],
                                        lhsT=w_sb[32 * r:32 * r + 12,
                                                  khw * 32:(khw + 1) * 32],
                                        rhs=quad[32 * r:32 * r + 12,
                                                 s0:s0 + CHUNK],
                                        start=(khw == 0),
                                        stop=(khw == 8),
                                        tile_position=(32 * r, 32 * c),
                                        skip_group_check=True,
                                    )
                        width = npairs * CHUNK
                        if g == 0:
                            nc.vector.tensor_copy(running[cgl][:, 0:width],
                                                  ps[:, 0:width])
                        else:
                            nc.vector.tensor_tensor(
                                out=running[cgl][:, 0:width],
                                in0=ps[:, 0:width],
                                in1=running[cgl][:, 0:width],
                                op=mybir.AluOpType.min,
                            )
                # tail: two chunk groups merge into one [128, 4096] tile with
                # (delta, pair) in the free dim (DMA crosses partitions),
                # then one reduce_min + softmax on full 128 lanes.
                # t2 partitions: p = k*64 + c*16 + co (k = cgl parity).
                for cp in range(NCGL // 2):
                    t2 = sm_pool.tile([128, 8 * CHUNK], F32, tag="t2")
                    for k in range(2):
                        cgl = 2 * cp + k
                        for dlt in range(2):
                            for c in range(4):
                                nc.sync.dma_start(
                                    t2[64 * k + 16 * c:64 * k + 16 * c + 16,
                                       2048 * dlt:2048 * (dlt + 1)],
                                    running[cgl][32 * c + 16 * dlt:
                                                 32 * c + 16 * dlt + 16, :])
                    mins = sm_pool.tile([128, CHUNK], F32, tag="mins")
                    nc.vector.tensor_reduce(
                        out=mins[:, :],
                        in_=t2[:, :].rearrange("p (dr s) -> p s dr", s=CHUNK),
                        axis=mybir.AxisListType.X,
                        op=mybir.AluOpType.min,
                    )
                    expt = exp_pool.tile([128, CHUNK], F32, tag="exp")
                    nc.scalar.activation(
                        expt[:, :], mins[:, :],
                        mybir.ActivationFunctionType.Exp,
                        bias=bias_sb[:, :], scale=1.0)
                    esum = psum_pool.tile([8, CHUNK], F32, tag="big")
                    nc.tensor.matmul(esum[:, :], lhsT=ones_sb[:, :],
                                     rhs=expt[:, :], start=True, stop=True)
                    rec = sm_pool.tile([8, CHUNK], F32, tag="rec")
                    nc.vector.reciprocal(rec[:, :], esum[:, :])
                    # broadcast rec rows over their 16-co partition groups
                    # via a K=8 ones matmul (PE does the partition fan-out)
                    rb = psum_pool.tile([128, CHUNK], F32, tag="big")
                    nc.tensor.matmul(rb[:, :], lhsT=ones_bc_sb[:, :],
                                     rhs=rec[:, :], start=True, stop=True)
                    soft = sm_pool.tile([128, CHUNK], F32, tag="soft")
                    nc.vector.tensor_tensor(
                        out=soft[:, :], in0=rb[:, :], in1=expt[:, :],
                        op=mybir.AluOpType.mult)
                    # emit valid rows: h = h0 + (2cp+k)*16 + c*4 + i
                    for k in range(2):
                        hbase = h0 + (2 * cp + k) * 16
                        for c in range(4):
                            hc = hbase + 4 * c
                            ni = min(4, HOUT - hc)
                            if ni <= 0:
                                continue
                            src = soft[64 * k + 16 * c:64 * k + 16 * c + 16,
                                       :].rearrange(
                                "co (i w) -> co i w", i=4)[:, 0:ni, 0:WOUT]
                            dst = y_ap[n, :, hc:hc + ni, :]
                            nc.sync.dma_start(dst, src)


def _compile():
    if "nc" in _COMPILED:
        return _COMPILED["nc"]
    nc = bacc.Bacc("TRN2", target_bir_lowering=False, debug=False,
                   num_devices=N_CORES)
    with tile.TileContext(nc) as tc:
        _emit_kernel(tc)
    nc.compile()
    _COMPILED["nc"] = nc
    return nc


def kernel(x, conv_weight, conv_bias):
    x = np.asarray(x, dtype=np.float32)
    conv_weight = np.asarray(conv_weight, dtype=np.float32)
    conv_bias = np.asarray(conv_bias, dtype=np.float32)

    xp = np.ascontiguousarray(x.transpose(0, 2, 1, 3, 4))  # [N, D, C, H, W]
    w_sb = _build_weight_blocks(conv_weight)
    bias_sb = _build_bias128(conv_bias)
    ones_sb = _build_ones()

    nc = _compile()
    in_maps = []
    for i in range(N_CORES):
        in_maps.append({
            "x": np.ascontiguousarray(xp[NB * i:NB * (i + 1)]).astype(ml_dtypes.bfloat16),
            "w": w_sb.astype(ml_dtypes.bfloat16),
            "bias": bias_sb,
            "ones": ones_sb,
            "ones_bc": _build_ones_bc(),
        })
    res = bass_utils.run_bass_kernel_spmd(
        nc, in_maps, core_ids=list(range(N_CORES)),
        trace=bool(int(os.environ.get("KERNEL_TRACE", "0"))),
    )
    _COMPILED["last_results"] = res
    out = np.concatenate([res.results[i]["y"] for i in range(N_CORES)], axis=0)
    return out


if __name__ == "__main__":
    _compile()
    print("build OK")



# revision 14
# speedup vs baseline: 2.3397x; 1.0396x over previous
"""Trainium2 Bass kernel for: Conv3d(3->16, k=3x3x3, VALID) + bias -> min over
depth -> softmax over channels.

Input  x: (16, 3, 32, 128, 128) f32   [N, C_in, D, H, W]
Weight w: (16, 3, 3, 3, 3) f32        [C_out, C_in, kD, kH, kW]
Bias   b: (16,) f32
Output  : (16, 16, 126, 126) f32      [N, C_out, H_out, W_out]

Data-parallel over batch: 2 batches per core x 8 cores. Per core:

  - x stored per (batch, h-half) as one [128, 8768] bf16 tile: strip r
    (partition quadrant 32r) holds 30 rows = (10 input depths 8r..8r+9) x
    (ci 3); free dim = local (h, w) flattened (66 or 64 h-rows + pad).
  - Conv as 4 row-packed matmuls per (chunk, khw): tile r = [K<=30, M=128,
    N=512] at tile_position (32r, 0); M = 8 local douts x 16 co; 9
    accumulating MMs over (kh,kw) with free-dim-shifted rhs (stride-1 conv
    == same spatial layout + offset koff). Weight col block[(dl,ci),
    (dll,co)] = w[co,ci,dl-dll,kh,kw]. PSUM supertile [128, 4*512]:
    bank r = strip r's 8 douts.
  - Strip 3 douts 30,31 don't exist: their weight cols are 0 except a
    BIG=32768 entry at khw=0 against a constant-1.0 rhs row, so those psum
    lanes hold +32768 and never win the depth-min.
  - Depth-min: one DVE tensor_reduce(min) over the 4 banks -> [128, 512],
    then a 128->64->32->16 partition tree-min on GpSimd into a [128, 512]
    collector (8 chunks x 16 co).
  - Softmax over co per 8-chunk group: ACT exp (bias fused; min(y)+b ==
    min(y+b)), PE ones-matmul for co-sums, DVE reciprocal, PE broadcast
    matmul, DVE multiply, DMA out valid rows.
"""

import os
import sys

sys.path.insert(0, "/opt/trn_rl_repo")

import numpy as np
import ml_dtypes

import concourse.bass as bass
import concourse.bacc as bacc
import concourse.tile as tile
import concourse.mybir as mybir
from concourse import bass_utils

F32 = mybir.dt.float32
BF16 = mybir.dt.bfloat16

N_CORES = 8
NB = 2           # batches per core
CI = 3
D = 32
H = 128
W = 128
CO = 16
CHUNK = 512
HOUT = 126
WOUT = 126
PAD = 320
QF = 66 * W + PAD  # quad tile free size (worst case hh=0)
BIG = 32768.0

_COMPILED = {}


def _strip_depths(r):
    return 10 if r < 3 else 8


def _kr(r, khw):
    if r < 3:
        return 30
    return 25 if khw == 0 else 24


def _build_weight_blocks(conv_weight):
    """[128, 9*128]: strip r rows 32r+(3*dl+ci); col khw*128 + dll*16 + co
    = w[co, ci, dl-dll, kh, kw] (0 outside kd range / dout>=30). Row 120
    (strip 3 local 24) carries BIG at khw=0 for the dout 30/31 lanes."""
    wb = np.zeros((128, 9 * 128), dtype=np.float32)
    for r in range(4):
        for dl in range(_strip_depths(r)):
            for ci in range(CI):
                row = 32 * r + 3 * dl + ci
                for khw in range(9):
                    kh, kw = khw // 3, khw % 3
                    for dll in range(8):
                        kd = dl - dll
                        if 8 * r + dll < 30 and 0 <= kd <= 2:
                            wb[row, khw * 128 + dll * 16:
                               khw * 128 + dll * 16 + 16] = \
                                conv_weight[:, ci, kd, kh, kw]
    for dll in (6, 7):
        wb[120, dll * 16:dll * 16 + 16] = BIG
    return wb.astype(ml_dtypes.bfloat16)


def _build_ones():
    """[128, 8]: col j sums partitions {16j + co : co}."""
    ones = np.zeros((128, 8), dtype=np.float32)
    for p in range(128):
        ones[p, p // 16] = 1.0
    return ones


def _build_ones_bc():
    """[8, 128]: transpose -- broadcasts row j over its 16-partition group."""
    return np.ascontiguousarray(_build_ones().T)


def _build_bias128(conv_bias):
    """[128, 1]: partition 16j + co -> bias[co]."""
    b = np.zeros((128, 1), dtype=np.float32)
    for p in range(128):
        b[p, 0] = conv_bias[p % 16]
    return b


def _emit_kernel(tc):
    nc = tc.nc
    x_ap = nc.dram_tensor("x", [NB, D, CI, H, W], BF16,
                          kind="ExternalInput").ap()
    w_ap = nc.dram_tensor("w", [128, 9 * 128], BF16,
                          kind="ExternalInput").ap()
    bias_ap = nc.dram_tensor("bias", [128, 1], F32, kind="ExternalInput").ap()
    ones_ap = nc.dram_tensor("ones", [128, 8], F32, kind="ExternalInput").ap()
    ones_bc_ap = nc.dram_tensor("ones_bc", [8, 128], F32,
                                kind="ExternalInput").ap()
    y_ap = nc.dram_tensor("y", [NB, CO, HOUT, WOUT], F32,
                          kind="ExternalOutput").ap()

    from contextlib import ExitStack

    with ExitStack() as ctx:
        const_pool = ctx.enter_context(tc.tile_pool(name="const", bufs=1))
        in_pool = ctx.enter_context(tc.tile_pool(name="in", bufs=2))
        m4w_pool = ctx.enter_context(tc.tile_pool(name="m4w", bufs=2))
        sm_pool = ctx.enter_context(tc.tile_pool(name="sm", bufs=2))
        psum_pool = ctx.enter_context(tc.tile_pool(name="ps", bufs=2,
                                                   space="PSUM"))

        w_sb = const_pool.tile([128, 9 * 128], BF16, tag="w")
        nc.sync.dma_start(w_sb[:, :], w_ap[:, :])
        bias_sb = const_pool.tile([128, 1], F32, tag="bias")
        nc.sync.dma_start(bias_sb[:, :], bias_ap[:, :])
        ones_sb = const_pool.tile([128, 8], F32, tag="ones")
        nc.sync.dma_start(ones_sb[:, :], ones_ap[:, :])
        ones_bc_sb = const_pool.tile([8, 128], F32, tag="onesbc")
        nc.sync.dma_start(ones_bc_sb[:, :], ones_bc_ap[:, :])

        for n in range(NB):
            for hh in range(2):
                h0 = 64 * hh
                hrows = 66 if hh == 0 else 64
                quad = in_pool.tile([128, QF], BF16, tag="quad")
                nc.gpsimd.memset(quad[:, hrows * W:QF], 0.0)
                # rows 96..127 <- 1.0; strip-3 DMA overwrites 96..119, leaving
                # the constant-1.0 row 120 for the BIG sentinel matmul column.
                nc.gpsimd.memset(quad[96:128, 0:hrows * W], 1.0)
                for r in range(4):
                    nd = _strip_depths(r)
                    src = x_ap[n, 8 * r:8 * r + nd, :,
                               h0:h0 + hrows, :].rearrange(
                        "d c h w -> (d c) (h w)")
                    nc.sync.dma_start(
                        quad[32 * r:32 * r + 3 * nd, 0:hrows * W], src)
                for q in range(2):
                    m4w = m4w_pool.tile([128, 8 * CHUNK], F32, tag="m4w")
                    for j in range(8):
                        m = 8 * q + j
                        s0 = CHUNK * m
                        ps = psum_pool.tile([128, 4 * CHUNK], F32, tag="big")
                        for khw in range(9):
                            kh, kw = khw // 3, khw % 3
                            koff = kh * W + kw
                            for r in range(4):
                                kr = _kr(r, khw)
                                nc.tensor.matmul(
                                    ps[:, r * CHUNK:(r + 1) * CHUNK],
                                    lhsT=w_sb[32 * r:32 * r + kr,
                                              khw * 128:(khw + 1) * 128],
                                    rhs=quad[32 * r:32 * r + kr,
                                             s0 + koff:s0 + koff + CHUNK],
                                    start=(khw == 0),
                                    stop=(khw == 8),
                                    tile_position=(32 * r, 0),
                                    skip_group_check=True,
                                )
                        nc.vector.tensor_reduce(
                            out=m4w[:, j * CHUNK:(j + 1) * CHUNK],
                            in_=ps[:, :].rearrange("p (r s) -> p s r",
                                                   s=CHUNK),
                            axis=mybir.AxisListType.X,
                            op=mybir.AluOpType.min,
                        )
                    # fold the 8 dout_local groups (partition tree-min).
                    # Engine TTs need equal SBUF partition bases, so shift
                    # the upper half down via SBUF-to-SBUF DMA each level.
                    sh = sm_pool.tile([64, 8 * CHUNK], F32, tag="sh")
                    nc.sync.dma_start(sh[0:64, :], m4w[64:128, :])
                    nc.vector.tensor_tensor(
                        out=m4w[0:64, :], in0=m4w[0:64, :], in1=sh[0:64, :],
                        op=mybir.AluOpType.min)
                    nc.sync.dma_start(sh[0:32, :], m4w[32:64, :])
                    nc.vector.tensor_tensor(
                        out=m4w[0:32, :], in0=m4w[0:32, :], in1=sh[0:32, :],
                        op=mybir.AluOpType.min)
                    nc.sync.dma_start(sh[0:16, :], m4w[16:32, :])
                    nc.vector.tensor_tensor(
                        out=m4w[0:16, :], in0=m4w[0:16, :], in1=sh[0:16, :],
                        op=mybir.AluOpType.min)
                    coll = sm_pool.tile([128, CHUNK], F32, tag="coll")
                    for j in range(8):
                        nc.sync.dma_start(
                            coll[16 * j:16 * j + 16, :],
                            m4w[0:16, j * CHUNK:(j + 1) * CHUNK])
                    expt = sm_pool.tile([128, CHUNK], F32, tag="exp")
                    nc.scalar.activation(
                        expt[:, :], coll[:, :],
                        mybir.ActivationFunctionType.Exp,
                        bias=bias_sb[:, :], scale=1.0)
                    esum = psum_pool.tile([8, CHUNK], F32, tag="big")
                    nc.tensor.matmul(esum[:, :], lhsT=ones_sb[:, :],
                                     rhs=expt[:, :], start=True, stop=True)
                    rec = sm_pool.tile([8, CHUNK], F32, tag="rec")
                    nc.vector.reciprocal(rec[:, :], esum[:, :])
                    rb = psum_pool.tile([128, CHUNK], F32, tag="big")
                    nc.tensor.matmul(rb[:, :], lhsT=ones_bc_sb[:, :],
                                     rhs=rec[:, :], start=True, stop=True)
                    soft = sm_pool.tile([128, CHUNK], F32, tag="soft")
                    nc.vector.tensor_tensor(
                        out=soft[:, :], in0=rb[:, :], in1=expt[:, :],
                        op=mybir.AluOpType.mult)
                    for j in range(8):
                        m = 8 * q + j
                        hc = h0 + 4 * m
                        ni = min(4, HOUT - hc)
                        if ni <= 0:
                            continue
                        src = soft[16 * j:16 * j + 16, :].rearrange(
                            "co (i w) -> co i w", i=4)[:, 0:ni, 0:WOUT]
                        nc.sync.dma_start(y_ap[n, :, hc:hc + ni, :], src)


def _compile():
    if "nc" in _COMPILED:
        return _COMPILED["nc"]
    nc = bacc.Bacc("TRN2", target_bir_lowering=False, debug=False,
                   num_devices=N_CORES)
    with tile.TileContext(nc) as tc:
        _emit_kernel(tc)
    nc.compile()
    _COMPILED["nc"] = nc
    return nc


def kernel(x, conv_weight, conv_bias):
    x = np.asarray(x, dtype=np.float32)
    conv_weight = np.asarray(conv_weight, dtype=np.float32)
    conv_bias = np.asarray(conv_bias, dtype=np.float32)

    xp = np.ascontiguousarray(
        x.transpose(0, 2, 1, 3, 4)).astype(ml_dtypes.bfloat16)  # [N,D,C,H,W]
    w_sb = _build_weight_blocks(conv_weight)
    bias_sb = _build_bias128(conv_bias)
    ones_sb = _build_ones()
    ones_bc_sb = _build_ones_bc()

    nc = _compile()
    in_maps = []
    for i in range(N_CORES):
        in_maps.append({
            "x": np.ascontiguousarray(xp[NB * i:NB * (i + 1)]),
            "w": w_sb,
            "bias": bias_sb,
            "ones": ones_sb,
            "ones_bc": ones_bc_sb,
        })
    res = bass_utils.run_bass_kernel_spmd(
        nc, in_maps, core_ids=list(range(N_CORES)),
        trace=bool(int(os.environ.get("KERNEL_TRACE", "0"))),
    )
    _COMPILED["last_results"] = res
    out = np.concatenate([res.results[i]["y"] for i in range(N_CORES)], axis=0)
    return out


if __name__ == "__main__":
    _compile()
    print("build OK")
